# revision 26
# baseline (speedup 1.0000x reference)
"""Trainium2 Bass kernel for nn_ComputeVecLoss (vector loss over keypoint graphs).

Math (per batch b):
  For every keypoint pair (i>j) sample 5 points on the segment; cdis = mean
  over the 5 points of the min squared distance to the 4096 gt points; an edge
  exists when cdis < 1e-3.  Loss = sum over edges of |u_i.u_j| / (|u_i||u_j|)
  divided by (1 + edge count), u_k = p0 - p_k.

Key structure:
  * Each batch needs only 425 unique query points (17 endpoints + 136*3
    interiors) instead of 17*17*5.
  * d2(r,m) = |K_r|^2 + |g_m|^2 - 2 K_r.g_m comes out of ONE TensorEngine
    contraction of depth 8: kaugT rows [k2_b0, k2_b1, -2Kx0, -2Ky0, blk0,
    -2Kx1, -2Ky1, blk1] against gaug rows [1, 1, gx0, gy0, g2_0-1/4, gx1,
    gy1, g2_1-1/4].  The k2 rows are built on device and land on PSUM
    partitions 0-1 so no partition-shifting copies are needed.
  * The query rows are PERMUTED so that the pmin SBUF tile [128, 7] is
    directly consumable: cols 0-2 hold batch-0 triples (pair p ->
    partition p), cols 4-6 hold batch-1 triples, and col 3 holds the 34
    endpoints (partitions 0-33) plus the 16 leftover triples (partitions
    34-81).  cdis5 is then a free-axis reduce plus selector matmuls -- the
    whole epilog runs on-chip with zero DRAM gathers.
  * The min over m=4096 is split between the Scalar engine (PSUM->SBUF bf16
    evacuation) and the Vector engine (f32 PSUM reduces + bf16 min-tree,
    bf16 tensor_tensor runs at 2x).

Sharding: batch dim 16 -> 8 cores x 2 batches.  Each core returns
[sum(cos), edge_count]; the host combines and divides.
"""

import os
import sys

for _p in ("/opt/trn_rl_repo",):
    if os.path.isdir(_p) and _p not in sys.path:
        sys.path.append(_p)

import numpy as np

B, N, D = 16, 17, 2
M = 4096
COUNT = 5
MAXDIS = 1e-3
EPS_ABS = 1e-5
TSOFT = 8e-5           # softmin temperature
LNC = 34.657359028     # ln(2^50) prescale keeps es well inside fp32 normals
LN2 = 0.6931471805599453
N_CORES = 8
BPC = B // N_CORES          # batches per core
NPAIR = N * (N - 1) // 2    # 136
ROWS2 = BPC * (N + 3 * NPAIR)  # 850 rows per core
RTILES = 7
RPAD = RTILES * 128         # 896
CONTR = 8                   # contraction depth of the hot matmul
PAIR2 = BPC * NPAIR         # 272 pairs per core
NLEFT = NPAIR - 128         # 8 leftover pairs per batch
SROWS = 2 * N + 2 * 3 * NLEFT  # 82 selector rows (34 endpoints + 48 slots)
GROUPS = [(0, 128), (128, 128), (256, 2 * NLEFT)]

PAIRS = [(i, j) for i in range(1, N) for j in range(i)]


def _row_endpoint(b, i):
    return 384 + N * b + i


def _row_triple(b, p, k):
    if p < 128:
        return 128 * k + p if b == 0 else 128 * (4 + k) + p
    q = p - 128
    return 384 + 2 * N + 3 * (NLEFT * b + q) + k


# constants blob column layout: [36, BLOBW]
CT0 = 0                     # ct            [36, 896]
WTC = CT0 + RPAD            # wt            [34, 34]
BOC = WTC + 2 * N           # blockones     [8, 2]
PAC = BOC + BPC             # p1aug         [36, 8]
PBC = PAC + CONTR           # p1_both       [34, 2]
BLOBW = PBC + D


def _constants():
    blob = np.zeros((2 * (N + 1), BLOBW), np.float32)
    ct = blob[:, CT0:CT0 + RPAD]
    s = np.zeros((SROWS, 2, PAIR2), np.float32)
    for b in range(BPC):
        base_c = (N + 1) * b
        for i in range(N):
            r = _row_endpoint(b, i)
            ct[base_c + i, r] = -2.0
            ct[base_c + N, r] = 1.0
        for p, (i, j) in enumerate(PAIRS):
            for k in range(3):
                t = 0.25 * (k + 1)
                r = _row_triple(b, p, k)
                ct[base_c + i, r] = -2.0 * t
                ct[base_c + j, r] = -2.0 * (1.0 - t)
                ct[base_c + N, r] = 1.0
        for p, (i, j) in enumerate(PAIRS):
            if p < 128:
                P = 128 * b + p
            else:
                P = 256 + NLEFT * b + (p - 128)
                for k in range(3):
                    s[2 * N + 3 * (NLEFT * b + p - 128) + k, 0, P] = 1.0
            s[N * b + i, 0, P] = 1.0
            s[N * b + j, 1, P] = 1.0
        for m in range(N):
            blob[N * b, WTC + N * b + m] += 1.0
            blob[N * b + m, WTC + N * b + m] -= 1.0
        blob[2 + 3 * b:5 + 3 * b, BOC + b] = 1.0
    return blob, np.ascontiguousarray(s.transpose(1, 2, 0).reshape(
        2 * PAIR2, SROWS).T)


_CONSTS = None
_COMPILED = None


def _get_consts():
    global _CONSTS
    if _CONSTS is None:
        _CONSTS = _constants()
    return _CONSTS


def _build():
    import concourse.bass as bass
    import concourse.bacc as bacc
    import concourse.tile as tile
    from concourse import mybir

    f32 = mybir.dt.float32
    f32r = mybir.dt.float32r
    bf16 = mybir.dt.bfloat16
    i32 = mybir.dt.int32
    Alu = mybir.AluOpType
    Act = mybir.ActivationFunctionType
    X = mybir.AxisListType.X

    nc = bacc.Bacc("TRN2", target_bir_lowering=False, debug=False,
                   num_devices=N_CORES)

    blob_d = nc.dram_tensor("blob", [2 * (N + 1), BLOBW], f32r,
                            kind="ExternalInput").ap()
    s_d = nc.dram_tensor("s", [SROWS, 2 * PAIR2], f32,
                         kind="ExternalInput").ap()
    gtt_d = nc.dram_tensor("gtt", [CONTR, M], f32r, kind="ExternalInput").ap()
    out_d = nc.dram_tensor("out", [2], f32, kind="ExternalOutput").ap()

    with tile.TileContext(nc) as tc:
        with (
            tc.tile_pool(name="singles", bufs=1) as singles,
            tc.tile_pool(name="work", bufs=2) as work,
            tc.tile_pool(name="psum", bufs=4, space="PSUM") as psum,
            tc.tile_pool(name="dram", bufs=1, space="DRAM") as dram,
        ):
            MK = M // 128  # 32

            # ---- inputs on two parallel HWDGE queues ----------------------
            blob_sb = singles.tile([2 * (N + 1), BLOBW], f32r)
            nc.sync.dma_start(out=blob_sb[:], in_=blob_d[:])
            gaug = singles.tile([CONTR, M], f32r)
            nc.scalar.dma_start(out=gaug[:], in_=gtt_d[:])
            s_sb = singles.tile([SROWS, 2 * PAIR2], f32)
            nc.sync.dma_start(out=s_sb[:], in_=s_d[:])

            ct_sb = blob_sb[:, CT0:CT0 + RPAD]
            wt_sb = blob_sb[0:2 * N, WTC:WTC + 2 * N]
            bones = blob_sb[0:CONTR, BOC:BOC + BPC]
            p1aug = blob_sb[:, PAC:PAC + CONTR]
            p1b = blob_sb[0:2 * N, PBC:PBC + D]

            eps_sb = singles.tile([2 * N, 1], f32)
            nc.gpsimd.memset(eps_sb[:], float(D * EPS_ABS))
            warm = work.tile([1, 1], f32, tag="warm")
            nc.scalar.activation(out=warm[:], in_=eps_sb[0:1, :],
                                 func=Act.Sqrt)

            # ---- kaugT [8, 896]: rows 2-7 from the ct matmul; rows 0-1 are
            #      0.25*(4|K_b|^2 + blk) built from the squared rows --------
            kaugT = singles.tile([CONTR, RPAD], f32r)
            sqk = singles.tile([CONTR, RPAD], f32r)
            CHUNKS = [(0, 512), (512, RPAD - 512)]
            kps = []
            for c0, cw in CHUNKS:
                kp = psum.tile([CONTR, cw], f32, tag="hot")
                nc.tensor.matmul(kp[:], p1aug, ct_sb[:, c0:c0 + cw],
                                 start=True, stop=True)
                kps.append(kp)
            up = psum.tile([2 * N, D], f32, tag="hot")
            nc.tensor.matmul(up[:], wt_sb, p1b, start=True, stop=True)
            for (c0, cw), kp in zip(CHUNKS, kps):
                nc.scalar.activation(out=sqk[:, c0:c0 + cw], in_=kp[:],
                                     func=Act.Square)
            for (c0, cw), kp in zip(CHUNKS, kps):
                nc.scalar.copy(out=kaugT[:, c0:c0 + cw], in_=kp[:])
            k2ps = []
            for c0, cw in CHUNKS:
                k2p = psum.tile([BPC, cw], f32, tag="hot")
                nc.tensor.matmul(k2p[:], bones, sqk[:, c0:c0 + cw],
                                 start=True, stop=True)
                k2ps.append(k2p)
            for (c0, cw), k2p in zip(CHUNKS, k2ps):
                nc.scalar.activation(out=kaugT[0:BPC, c0:c0 + cw], in_=k2p[:],
                                     func=Act.Copy, scale=0.25)

            # ---- u vectors / |u| for the cosine epilog --------------------
            uext = singles.tile([SROWS, 4], f32)
            nc.gpsimd.memset(uext[:], 0.0)
            uf = work.tile([2 * N, 2], f32, tag="uf")
            nc.vector.tensor_copy(out=uf[:], in_=up[:])
            nc.vector.tensor_copy(out=uext[0:2 * N, 0:2], in_=uf[:])
            uscr = work.tile([2 * N, 2], f32, tag="u")
            a0 = work.tile([2 * N, 1], f32, tag="u2")
            nc.vector.tensor_mul(uscr[:], uf[:], uf[:])
            nc.vector.reduce_sum(out=a0[:], in_=uscr[:], axis=X)
            nc.scalar.activation(out=uext[0:2 * N, 2:3], in_=a0[:],
                                 func=Act.Sqrt, bias=eps_sb[:])
            nc.scalar.activation(out=warm[:], in_=eps_sb[0:1, :],
                                 func=Act.Exp)

            onescol = singles.tile([128, 1], f32)
            nc.gpsimd.memset(onescol[:], 1.0)
            cm_all = singles.tile([128, 6], f32)
            nc.gpsimd.memset(cm_all[:], 0.0)

            pmin = singles.tile([128, RTILES], f32)

            # ---- stage 5 chain, emitted per group once its pmin cols exist
            def emit_group_mm(g, g0, cnt):
                s1p = psum.tile([cnt, 4], f32, tag="hot")
                nc.tensor.matmul(s1p[:], s_sb[:, g0:g0 + cnt], uext[:],
                                 start=True, stop=True)
                s2p = psum.tile([cnt, 4], f32, tag="hot")
                nc.tensor.matmul(s2p[:], s_sb[:, PAIR2 + g0:PAIR2 + g0 + cnt],
                                 uext[:], start=True, stop=True)
                sb1 = work.tile([cnt, 4], f32, tag="sb1")
                sb2 = work.tile([cnt, 4], f32, tag="sb2")
                nc.scalar.copy(out=sb1[:], in_=s1p[:])
                nc.scalar.copy(out=sb2[:], in_=s2p[:])
                return sb1, sb2

            def emit_chain_a(g, g0, cnt, sb1, sb2):
                # only needs the selector outputs (not pmin)
                dscr = work.tile([cnt, 2], f32, tag="ds" + str(g))
                dot = work.tile([cnt, 4], f32, tag="dot" + str(g))
                nc.vector.tensor_mul(dscr[:], sb1[:, 0:2], sb2[:, 0:2])
                nc.vector.reduce_sum(out=dot[:, 0:1], in_=dscr[:], axis=X)
                nc.vector.tensor_reduce(out=dot[:, 1:2], in_=dot[:, 0:1],
                                        axis=X, op=Alu.max,
                                        apply_absolute_value=True)
                nc.vector.tensor_mul(dot[:, 2:3], sb1[:, 2:3], sb2[:, 2:3])
                rec = work.tile([cnt, 1], f32, tag="rec" + str(g))
                nc.vector.reciprocal(out=rec[:], in_=dot[:, 2:3])
                nc.vector.tensor_mul(dot[:, 3:4], dot[:, 1:2], rec[:])
                return dot

            def emit_chain_b(g, g0, cnt, sb1, sb2, dot):
                # pmin-dependent part
                c3 = work.tile([cnt, 4], f32, tag="c3")
                nc.vector.tensor_add(c3[:, 1:2], sb1[:, 3:4], sb2[:, 3:4])
                if g == 0:
                    nc.vector.tensor_reduce(out=c3[:, 0:1],
                                            in_=pmin[0:cnt, 0:3],
                                            axis=X, op=Alu.add)
                    nc.vector.tensor_add(c3[:, 2:3], c3[:, 1:2], c3[:, 0:1])
                elif g == 1:
                    nc.vector.tensor_reduce(out=c3[:, 0:1],
                                            in_=pmin[0:cnt, 4:7],
                                            axis=X, op=Alu.add)
                    nc.vector.tensor_add(c3[:, 2:3], c3[:, 1:2], c3[:, 0:1])
                else:
                    nc.vector.tensor_copy(out=c3[:, 2:3], in_=c3[:, 1:2])
                msk = work.tile([cnt, 1], f32, tag="msk")
                nc.vector.tensor_single_scalar(
                    out=msk[:], in_=c3[:, 2:3],
                    scalar=float(COUNT * MAXDIS), op=Alu.is_lt)
                nc.vector.tensor_copy(out=cm_all[0:cnt, 3 + g:4 + g],
                                      in_=msk[:])
                nc.vector.tensor_mul(cm_all[0:cnt, g:g + 1], dot[:, 3:4],
                                     msk[:])

            # ---- hot loop: d2 matmuls; min over m = hard (DVE) on banks
            #      A,B + exp-softmin (ACT Exp-accumulate) on banks C,D ------
            lnc_sb = singles.tile([128, 1], f32)
            nc.gpsimd.memset(lnc_sb[:], float(LNC))
            hmA = singles.tile([128, RTILES], f32)
            hmB = singles.tile([128, RTILES], f32)
            es = singles.tile([128, RTILES], f32)
            soft = singles.tile([128, RTILES], f32)
            for t in range(RTILES):
                wtile = kaugT[:, 128 * t:128 * (t + 1)]
                pA = psum.tile([128, 1024], f32, tag="hot")
                pB = psum.tile([128, 1024], f32, tag="hot")
                pC = psum.tile([128, 1024], f32, tag="hot")
                pD = psum.tile([128, 1024], f32, tag="hot")
                for h, ph in enumerate((pA, pB, pC, pD)):
                    for j in range(2):
                        nc.tensor.matmul(
                            ph[:, 512 * j:512 * (j + 1)], wtile,
                            gaug[:, 1024 * h + 512 * j:1024 * h + 512 * (j + 1)],
                            start=True, stop=True)
                junkC = work.tile([128, 1024], bf16, tag="jC")
                junkD = work.tile([128, 1024], bf16, tag="jD")
                eC = work.tile([128, 1], f32, tag="eC")
                eD = work.tile([128, 1], f32, tag="eD")
                nc.scalar.activation(out=junkC[:], in_=pC[:], func=Act.Exp,
                                     scale=float(-1.0 / TSOFT), bias=lnc_sb[:],
                                     accum_out=eC[:])
                nc.scalar.activation(out=junkD[:], in_=pD[:], func=Act.Exp,
                                     scale=float(-1.0 / TSOFT), bias=lnc_sb[:],
                                     accum_out=eD[:])
                nc.vector.tensor_reduce(out=hmA[:, t:t + 1], in_=pA[:],
                                        axis=X, op=Alu.min)
                nc.vector.tensor_reduce(out=hmB[:, t:t + 1], in_=pB[:],
                                        axis=X, op=Alu.min)
                nc.gpsimd.tensor_add(es[:, t:t + 1], eC[:], eD[:])
                # pmin col = min(hard, -T*ln2*floor(log2 es) + T*lnC):
                # exponent-only log via bit shift -- +-ln2 slop is far inside
                # the threshold margin and needs no ACT table
                eint = work.tile([128, 1], i32, tag="eint")
                ef = work.tile([128, 1], f32, tag="ef")
                nc.vector.tensor_single_scalar(
                    out=eint[:], in_=es[:, t:t + 1].bitcast(i32),
                    scalar=23, op=Alu.arith_shift_right)
                nc.vector.tensor_copy(out=ef[:], in_=eint[:])
                nc.vector.tensor_scalar(
                    out=soft[:, t:t + 1], in0=ef[:],
                    scalar1=float(-TSOFT * LN2),
                    scalar2=float(TSOFT * (127.0 * LN2 + LNC)),
                    op0=Alu.mult, op1=Alu.add)
                nc.vector.tensor_tensor(out=pmin[:, t:t + 1],
                                        in0=hmA[:, t:t + 1],
                                        in1=hmB[:, t:t + 1], op=Alu.min)
                nc.vector.tensor_tensor(out=pmin[:, t:t + 1],
                                        in0=pmin[:, t:t + 1],
                                        in1=soft[:, t:t + 1], op=Alu.min)
                if t == 3:
                    nc.vector.tensor_copy(out=uext[:, 3:4],
                                          in_=pmin[0:SROWS, 3:4])
                if t == 3:
                    g0sb = emit_group_mm(0, 0, 128)
                if t == 4:
                    g0dot = emit_chain_a(0, 0, 128, *g0sb)
                    emit_chain_b(0, 0, 128, *g0sb, g0dot)
                    g2sb = emit_group_mm(2, 256, 2 * NLEFT)
                if t == 5:
                    g2dot = emit_chain_a(2, 256, 2 * NLEFT, *g2sb)
                    emit_chain_b(2, 256, 2 * NLEFT, *g2sb, g2dot)
                    g1sb = emit_group_mm(1, 128, 128)
                if t == 6:
                    g1dot = emit_chain_a(1, 128, 128, *g1sb)
            emit_chain_b(1, 128, 128, *g1sb, g1dot)

            tot = psum.tile([1, 6], f32, tag="hot")
            nc.tensor.matmul(tot[:], onescol[:], cm_all[:],
                             start=True, stop=True)
            res = work.tile([1, 2], f32, tag="res")
            tot3 = tot[:].rearrange("p (j c) -> p j c", c=3)
            nc.vector.tensor_reduce(out=res[:], in_=tot3, axis=X, op=Alu.add)
            nc.sync.dma_start(out=out_d.rearrange("(a b) -> a b", a=1),
                              in_=res[:])

    nc.compile()
    return nc


def _make_in_maps(recon_points: np.ndarray, gt_points: np.ndarray):
    blob0, s = _get_consts()
    recon_points = np.ascontiguousarray(recon_points, np.float32)
    gt_points = np.ascontiguousarray(gt_points, np.float32)
    in_maps = []
    for k in range(N_CORES):
        blob = blob0.copy()
        for b in range(BPC):
            rec = recon_points[BPC * k + b]
            blob[(N + 1) * b:(N + 1) * b + N,
                 PAC + 2 + 3 * b:PAC + 4 + 3 * b] = rec
            blob[(N + 1) * b + N, PAC + 4 + 3 * b] = 1.0
            blob[N * b:N * b + N, PBC:PBC + 2] = rec
        gt_pair = gt_points[BPC * k:BPC * (k + 1)]          # [2, 4096, 2]
        gtt = np.empty((CONTR, M), np.float32)
        gtt[0:2] = 1.0
        gtt[2:4] = gt_pair[0].T
        gtt[4] = (gt_pair[0] * gt_pair[0]).sum(-1) - 0.25
        gtt[5:7] = gt_pair[1].T
        gtt[7] = (gt_pair[1] * gt_pair[1]).sum(-1) - 0.25
        in_maps.append({"blob": blob, "s": s, "gtt": gtt})
    return in_maps


def kernel(recon_points: np.ndarray, gt_points: np.ndarray) -> np.ndarray:
    from concourse.bass_utils import run_bass_kernel_spmd

    global _COMPILED
    if _COMPILED is None:
        _COMPILED = _build()
    nc = _COMPILED

    in_maps = _make_in_maps(recon_points, gt_points)
    res = run_bass_kernel_spmd(nc, in_maps, core_ids=list(range(N_CORES)))
    partials = np.stack([r["out"] for r in res.results])  # [8, 2]
    cos_sum = partials[:, 0].sum(dtype=np.float32)
    cnt = partials[:, 1].sum(dtype=np.float32)
    return np.float32(cos_sum / (np.float32(1.0) + cnt))


# revision 27
# speedup vs baseline: 1.0341x; 1.0341x over previous
"""Trainium2 Bass kernel for nn_ComputeVecLoss (vector loss over keypoint graphs).

Math (per batch b):
  For every keypoint pair (i>j) sample 5 points on the segment; cdis = mean
  over the 5 points of the min squared distance to the 4096 gt points; an edge
  exists when cdis < 1e-3.  Loss = sum over edges of |u_i.u_j| / (|u_i||u_j|)
  divided by (1 + edge count), u_k = p0 - p_k.

Key structure:
  * Each batch needs only 425 unique query points (17 endpoints + 136*3
    interiors) instead of 17*17*5.
  * d2(r,m) = |K_r|^2 + |g_m|^2 - 2 K_r.g_m comes out of ONE TensorEngine
    contraction of depth 8: kaugT rows [k2_b0, k2_b1, -2Kx0, -2Ky0, blk0,
    -2Kx1, -2Ky1, blk1] against gaug rows [1, 1, gx0, gy0, g2_0-1/4, gx1,
    gy1, g2_1-1/4].  The k2 rows are built on device and land on PSUM
    partitions 0-1 so no partition-shifting copies are needed.
  * The query rows are PERMUTED so that the pmin SBUF tile [128, 7] is
    directly consumable: cols 0-2 hold batch-0 triples (pair p ->
    partition p), cols 4-6 hold batch-1 triples, and col 3 holds the 34
    endpoints (partitions 0-33) plus the 16 leftover triples (partitions
    34-81).  cdis5 is then a free-axis reduce plus selector matmuls -- the
    whole epilog runs on-chip with zero DRAM gathers.
  * The min over m=4096 is split between the Scalar engine (PSUM->SBUF bf16
    evacuation) and the Vector engine (f32 PSUM reduces + bf16 min-tree,
    bf16 tensor_tensor runs at 2x).

Sharding: batch dim 16 -> 8 cores x 2 batches.  Each core returns
[sum(cos), edge_count]; the host combines and divides.
"""

import os
import sys

for _p in ("/opt/trn_rl_repo",):
    if os.path.isdir(_p) and _p not in sys.path:
        sys.path.append(_p)

import numpy as np

B, N, D = 16, 17, 2
M = 4096
COUNT = 5
MAXDIS = 1e-3
EPS_ABS = 1e-5
TSOFT = 8e-5           # softmin temperature
LNC = 34.657359028     # ln(2^50) prescale keeps es well inside fp32 normals
LN2 = 0.6931471805599453
N_CORES = 8
BPC = B // N_CORES          # batches per core
NPAIR = N * (N - 1) // 2    # 136
ROWS2 = BPC * (N + 3 * NPAIR)  # 850 rows per core
RTILES = 7
RPAD = RTILES * 128         # 896
CONTR = 8                   # contraction depth of the hot matmul
PAIR2 = BPC * NPAIR         # 272 pairs per core
NLEFT = NPAIR - 128         # 8 leftover pairs per batch
SROWS = 2 * N + 2 * 3 * NLEFT  # 82 selector rows (34 endpoints + 48 slots)
GROUPS = [(0, 128), (128, 128), (256, 2 * NLEFT)]

PAIRS = [(i, j) for i in range(1, N) for j in range(i)]


def _row_endpoint(b, i):
    return 384 + N * b + i


def _row_triple(b, p, k):
    if p < 128:
        return 128 * k + p if b == 0 else 128 * (4 + k) + p
    q = p - 128
    return 384 + 2 * N + 3 * (NLEFT * b + q) + k


# constants blob column layout: [36, BLOBW]
CT0 = 0                     # ct            [36, 896]
WTC = CT0 + RPAD            # wt            [34, 34]
BOC = WTC + 2 * N           # blockones     [8, 2]
PAC = BOC + BPC             # p1aug         [36, 8]
PBC = PAC + CONTR           # p1_both       [34, 2]
BLOBW = PBC + D


def _constants():
    blob = np.zeros((2 * (N + 1), BLOBW), np.float32)
    ct = blob[:, CT0:CT0 + RPAD]
    s = np.zeros((SROWS, 2, PAIR2), np.float32)
    for b in range(BPC):
        base_c = (N + 1) * b
        for i in range(N):
            r = _row_endpoint(b, i)
            ct[base_c + i, r] = -2.0
            ct[base_c + N, r] = 1.0
        for p, (i, j) in enumerate(PAIRS):
            for k in range(3):
                t = 0.25 * (k + 1)
                r = _row_triple(b, p, k)
                ct[base_c + i, r] = -2.0 * t
                ct[base_c + j, r] = -2.0 * (1.0 - t)
                ct[base_c + N, r] = 1.0
        for p, (i, j) in enumerate(PAIRS):
            if p < 128:
                P = 128 * b + p
            else:
                P = 256 + NLEFT * b + (p - 128)
                for k in range(3):
                    s[2 * N + 3 * (NLEFT * b + p - 128) + k, 0, P] = 1.0
            s[N * b + i, 0, P] = 1.0
            s[N * b + j, 1, P] = 1.0
        for m in range(N):
            blob[N * b, WTC + N * b + m] += 1.0
            blob[N * b + m, WTC + N * b + m] -= 1.0
        blob[2 + 3 * b:5 + 3 * b, BOC + b] = 1.0
    return blob, np.ascontiguousarray(s.transpose(1, 2, 0).reshape(
        2 * PAIR2, SROWS).T)


_CONSTS = None
_COMPILED = None


def _get_consts():
    global _CONSTS
    if _CONSTS is None:
        _CONSTS = _constants()
    return _CONSTS


def _build():
    import concourse.bass as bass
    import concourse.bacc as bacc
    import concourse.tile as tile
    from concourse import mybir

    f32 = mybir.dt.float32
    f32r = mybir.dt.float32r
    bf16 = mybir.dt.bfloat16
    i32 = mybir.dt.int32
    Alu = mybir.AluOpType
    Act = mybir.ActivationFunctionType
    X = mybir.AxisListType.X

    nc = bacc.Bacc("TRN2", target_bir_lowering=False, debug=False,
                   num_devices=N_CORES)

    blob_d = nc.dram_tensor("blob", [2 * (N + 1), BLOBW], f32r,
                            kind="ExternalInput").ap()
    s_d = nc.dram_tensor("s", [SROWS, 2 * PAIR2], f32,
                         kind="ExternalInput").ap()
    gtt_d = nc.dram_tensor("gtt", [CONTR, M], f32r, kind="ExternalInput").ap()
    out_d = nc.dram_tensor("out", [2], f32, kind="ExternalOutput").ap()

    with tile.TileContext(nc) as tc:
        with (
            tc.tile_pool(name="singles", bufs=1) as singles,
            tc.tile_pool(name="work", bufs=2) as work,
            tc.tile_pool(name="psum", bufs=4, space="PSUM") as psum,
            tc.tile_pool(name="dram", bufs=1, space="DRAM") as dram,
        ):
            MK = M // 128  # 32

            # ---- inputs on two parallel HWDGE queues ----------------------
            blob_sb = singles.tile([2 * (N + 1), BLOBW], f32r)
            nc.sync.dma_start(out=blob_sb[:], in_=blob_d[:])
            gaug = singles.tile([CONTR, M], f32r)
            nc.scalar.dma_start(out=gaug[:], in_=gtt_d[:])
            s_sb = singles.tile([SROWS, 2 * PAIR2], f32)
            nc.sync.dma_start(out=s_sb[:], in_=s_d[:])

            ct_sb = blob_sb[:, CT0:CT0 + RPAD]
            wt_sb = blob_sb[0:2 * N, WTC:WTC + 2 * N]
            bones = blob_sb[0:CONTR, BOC:BOC + BPC]
            p1aug = blob_sb[:, PAC:PAC + CONTR]
            p1b = blob_sb[0:2 * N, PBC:PBC + D]

            eps_sb = singles.tile([2 * N, 1], f32)
            nc.gpsimd.memset(eps_sb[:], float(D * EPS_ABS))
            warm = work.tile([1, 1], f32, tag="warm")
            nc.scalar.activation(out=warm[:], in_=eps_sb[0:1, :],
                                 func=Act.Sqrt)

            # ---- kaugT [8, 896]: rows 2-7 from the ct matmul; rows 0-1 are
            #      0.25*(4|K_b|^2 + blk) built from the squared rows --------
            kaugT = singles.tile([CONTR, RPAD], f32r)
            sqk = singles.tile([CONTR, RPAD], f32r)
            CHUNKS = [(0, 512), (512, RPAD - 512)]
            kps = []
            for c0, cw in CHUNKS:
                kp = psum.tile([CONTR, cw], f32, tag="hot")
                nc.tensor.matmul(kp[:], p1aug, ct_sb[:, c0:c0 + cw],
                                 start=True, stop=True)
                kps.append(kp)
            up = psum.tile([2 * N, D], f32, tag="hot")
            nc.tensor.matmul(up[:], wt_sb, p1b, start=True, stop=True)
            for (c0, cw), kp in zip(CHUNKS, kps):
                nc.scalar.activation(out=sqk[:, c0:c0 + cw], in_=kp[:],
                                     func=Act.Square)
            for (c0, cw), kp in zip(CHUNKS, kps):
                nc.scalar.copy(out=kaugT[:, c0:c0 + cw], in_=kp[:])
            k2ps = []
            for c0, cw in CHUNKS:
                k2p = psum.tile([BPC, cw], f32, tag="hot")
                nc.tensor.matmul(k2p[:], bones, sqk[:, c0:c0 + cw],
                                 start=True, stop=True)
                k2ps.append(k2p)
            for (c0, cw), k2p in zip(CHUNKS, k2ps):
                nc.scalar.activation(out=kaugT[0:BPC, c0:c0 + cw], in_=k2p[:],
                                     func=Act.Copy, scale=0.25)

            # ---- u vectors / |u| for the cosine epilog --------------------
            uext = singles.tile([SROWS, 4], f32)
            nc.gpsimd.memset(uext[:], 0.0)
            uf = work.tile([2 * N, 2], f32, tag="uf")
            nc.vector.tensor_copy(out=uf[:], in_=up[:])
            nc.vector.tensor_copy(out=uext[0:2 * N, 0:2], in_=uf[:])
            uscr = work.tile([2 * N, 2], f32, tag="u")
            a0 = work.tile([2 * N, 1], f32, tag="u2")
            nc.vector.tensor_mul(uscr[:], uf[:], uf[:])
            nc.vector.reduce_sum(out=a0[:], in_=uscr[:], axis=X)
            nc.scalar.activation(out=uext[0:2 * N, 2:3], in_=a0[:],
                                 func=Act.Sqrt, bias=eps_sb[:])
            nc.scalar.activation(out=warm[:], in_=eps_sb[0:1, :],
                                 func=Act.Exp)

            onescol = singles.tile([128, 1], f32)
            nc.gpsimd.memset(onescol[:], 1.0)
            cm_all = singles.tile([128, 6], f32)
            nc.gpsimd.memset(cm_all[:], 0.0)

            pmin = singles.tile([128, RTILES], f32)

            # ---- stage 5 chain, emitted per group once its pmin cols exist
            def emit_group_mm(g, g0, cnt):
                s1p = psum.tile([cnt, 4], f32, tag="hot")
                nc.tensor.matmul(s1p[:], s_sb[:, g0:g0 + cnt], uext[:],
                                 start=True, stop=True)
                s2p = psum.tile([cnt, 4], f32, tag="hot")
                nc.tensor.matmul(s2p[:], s_sb[:, PAIR2 + g0:PAIR2 + g0 + cnt],
                                 uext[:], start=True, stop=True)
                sb1 = work.tile([cnt, 4], f32, tag="sb1")
                sb2 = work.tile([cnt, 4], f32, tag="sb2")
                nc.scalar.copy(out=sb1[:], in_=s1p[:])
                nc.scalar.copy(out=sb2[:], in_=s2p[:])
                return sb1, sb2

            def emit_chain_a(g, g0, cnt, sb1, sb2):
                # only needs the selector outputs (not pmin)
                dscr = work.tile([cnt, 2], f32, tag="ds" + str(g))
                dot = work.tile([cnt, 4], f32, tag="dot" + str(g))
                nc.vector.tensor_mul(dscr[:], sb1[:, 0:2], sb2[:, 0:2])
                nc.vector.reduce_sum(out=dot[:, 0:1], in_=dscr[:], axis=X)
                nc.vector.tensor_reduce(out=dot[:, 1:2], in_=dot[:, 0:1],
                                        axis=X, op=Alu.max,
                                        apply_absolute_value=True)
                nc.vector.tensor_mul(dot[:, 2:3], sb1[:, 2:3], sb2[:, 2:3])
                rec = work.tile([cnt, 1], f32, tag="rec" + str(g))
                nc.vector.reciprocal(out=rec[:], in_=dot[:, 2:3])
                nc.vector.tensor_mul(dot[:, 3:4], dot[:, 1:2], rec[:])
                return dot

            def emit_chain_b(g, g0, cnt, sb1, sb2, dot):
                # pmin-dependent part
                c3 = work.tile([cnt, 4], f32, tag="c3")
                nc.vector.tensor_add(c3[:, 1:2], sb1[:, 3:4], sb2[:, 3:4])
                if g == 0:
                    nc.vector.tensor_reduce(out=c3[:, 0:1],
                                            in_=pmin[0:cnt, 0:3],
                                            axis=X, op=Alu.add)
                    nc.vector.tensor_add(c3[:, 2:3], c3[:, 1:2], c3[:, 0:1])
                elif g == 1:
                    nc.vector.tensor_reduce(out=c3[:, 0:1],
                                            in_=pmin[0:cnt, 4:7],
                                            axis=X, op=Alu.add)
                    nc.vector.tensor_add(c3[:, 2:3], c3[:, 1:2], c3[:, 0:1])
                else:
                    nc.vector.tensor_copy(out=c3[:, 2:3], in_=c3[:, 1:2])
                msk = work.tile([cnt, 1], f32, tag="msk")
                nc.vector.tensor_single_scalar(
                    out=msk[:], in_=c3[:, 2:3],
                    scalar=float(COUNT * MAXDIS), op=Alu.is_lt)
                nc.vector.tensor_copy(out=cm_all[0:cnt, 3 + g:4 + g],
                                      in_=msk[:])
                nc.vector.tensor_mul(cm_all[0:cnt, g:g + 1], dot[:, 3:4],
                                     msk[:])

            # ---- hot loop: d2 matmuls; min over m = hard (DVE) on banks
            #      A,B + exp-softmin (ACT Exp-accumulate) on banks C,D ------
            lnc_sb = singles.tile([128, 1], f32)
            nc.gpsimd.memset(lnc_sb[:], float(LNC))
            hmA = singles.tile([128, RTILES], f32)
            hmB = singles.tile([128, RTILES], f32)
            es = singles.tile([128, RTILES], f32)
            soft = singles.tile([128, RTILES], f32)
            for t in range(RTILES):
                wtile = kaugT[:, 128 * t:128 * (t + 1)]
                pA = psum.tile([128, 1024], f32, tag="hot")
                pB = psum.tile([128, 1024], f32, tag="hot")
                pC = psum.tile([128, 1024], f32, tag="hot")
                pD = psum.tile([128, 1024], f32, tag="hot")
                for h, ph in enumerate((pA, pB, pC, pD)):
                    for j in range(2):
                        nc.tensor.matmul(
                            ph[:, 512 * j:512 * (j + 1)], wtile,
                            gaug[:, 1024 * h + 512 * j:1024 * h + 512 * (j + 1)],
                            start=True, stop=True)
                junkC = work.tile([128, 1024], bf16, tag="jC")
                junkD = work.tile([128, 1024], bf16, tag="jD")
                eC = work.tile([128, 1], f32, tag="eC")
                eD = work.tile([128, 1], f32, tag="eD")
                nc.scalar.activation(out=junkC[:], in_=pC[:], func=Act.Exp,
                                     scale=float(-1.0 / TSOFT), bias=lnc_sb[:],
                                     accum_out=eC[:])
                nc.scalar.activation(out=junkD[:], in_=pD[:], func=Act.Exp,
                                     scale=float(-1.0 / TSOFT), bias=lnc_sb[:],
                                     accum_out=eD[:])
                nc.vector.tensor_reduce(out=hmA[:, t:t + 1], in_=pA[:],
                                        axis=X, op=Alu.min)
                nc.vector.tensor_reduce(out=hmB[:, t:t + 1], in_=pB[:],
                                        axis=X, op=Alu.min)
                nc.gpsimd.tensor_add(es[:, t:t + 1], eC[:], eD[:])
                # pmin col = min(hard, -T*ln2*floor(log2 es) + T*lnC):
                # exponent-only log via bit shift -- +-ln2 slop is far inside
                # the threshold margin and needs no ACT table
                eint = work.tile([128, 1], i32, tag="eint")
                ef = work.tile([128, 1], f32, tag="ef")
                nc.vector.tensor_single_scalar(
                    out=eint[:], in_=es[:, t:t + 1].bitcast(i32),
                    scalar=23, op=Alu.arith_shift_right)
                nc.vector.tensor_copy(out=ef[:], in_=eint[:])
                nc.vector.tensor_scalar(
                    out=soft[:, t:t + 1], in0=ef[:],
                    scalar1=float(-TSOFT * LN2),
                    scalar2=float(TSOFT * (127.0 * LN2 + LNC)),
                    op0=Alu.mult, op1=Alu.add)
                nc.vector.tensor_tensor(out=pmin[:, t:t + 1],
                                        in0=hmA[:, t:t + 1],
                                        in1=hmB[:, t:t + 1], op=Alu.min)
                nc.vector.tensor_tensor(out=pmin[:, t:t + 1],
                                        in0=pmin[:, t:t + 1],
                                        in1=soft[:, t:t + 1], op=Alu.min)
                if t == 3:
                    nc.vector.tensor_copy(out=uext[:, 3:4],
                                          in_=pmin[0:SROWS, 3:4])
                if t == 4:
                    g0sb = emit_group_mm(0, 0, 128)
                    g0dot = emit_chain_a(0, 0, 128, *g0sb)
                    emit_chain_b(0, 0, 128, *g0sb, g0dot)
                if t == 5:
                    g2sb = emit_group_mm(2, 256, 2 * NLEFT)
                    g2dot = emit_chain_a(2, 256, 2 * NLEFT, *g2sb)
                    emit_chain_b(2, 256, 2 * NLEFT, *g2sb, g2dot)
                    g1sb = emit_group_mm(1, 128, 128)
                if t == 6:
                    g1dot = emit_chain_a(1, 128, 128, *g1sb)
            emit_chain_b(1, 128, 128, *g1sb, g1dot)

            tot = psum.tile([1, 6], f32, tag="hot")
            nc.tensor.matmul(tot[:], onescol[:], cm_all[:],
                             start=True, stop=True)
            res = work.tile([1, 2], f32, tag="res")
            tot3 = tot[:].rearrange("p (j c) -> p j c", c=3)
            nc.vector.tensor_reduce(out=res[:], in_=tot3, axis=X, op=Alu.add)
            nc.sync.dma_start(out=out_d.rearrange("(a b) -> a b", a=1),
                              in_=res[:])

    nc.compile()
    return nc


def _make_in_maps(recon_points: np.ndarray, gt_points: np.ndarray):
    blob0, s = _get_consts()
    recon_points = np.ascontiguousarray(recon_points, np.float32)
    gt_points = np.ascontiguousarray(gt_points, np.float32)
    in_maps = []
    for k in range(N_CORES):
        blob = blob0.copy()
        for b in range(BPC):
            rec = recon_points[BPC * k + b]
            blob[(N + 1) * b:(N + 1) * b + N,
                 PAC + 2 + 3 * b:PAC + 4 + 3 * b] = rec
            blob[(N + 1) * b + N, PAC + 4 + 3 * b] = 1.0
            blob[N * b:N * b + N, PBC:PBC + 2] = rec
        gt_pair = gt_points[BPC * k:BPC * (k + 1)]          # [2, 4096, 2]
        gtt = np.empty((CONTR, M), np.float32)
        gtt[0:2] = 1.0
        gtt[2:4] = gt_pair[0].T
        gtt[4] = (gt_pair[0] * gt_pair[0]).sum(-1) - 0.25
        gtt[5:7] = gt_pair[1].T
        gtt[7] = (gt_pair[1] * gt_pair[1]).sum(-1) - 0.25
        in_maps.append({"blob": blob, "s": s, "gtt": gtt})
    return in_maps


def kernel(recon_points: np.ndarray, gt_points: np.ndarray) -> np.ndarray:
    from concourse.bass_utils import run_bass_kernel_spmd

    global _COMPILED
    if _COMPILED is None:
        _COMPILED = _build()
    nc = _COMPILED

    in_maps = _make_in_maps(recon_points, gt_points)
    res = run_bass_kernel_spmd(nc, in_maps, core_ids=list(range(N_CORES)))
    partials = np.stack([r["out"] for r in res.results])  # [8, 2]
    cos_sum = partials[:, 0].sum(dtype=np.float32)
    cnt = partials[:, 1].sum(dtype=np.float32)
    return np.float32(cos_sum / (np.float32(1.0) + cnt))
